# revision 2
# baseline (speedup 1.0000x reference)
"""Nemotron-H top-k router kernel for Trainium2 (8 NeuronCores, SPMD data-parallel).

Strategy: shard the 16384 flattened tokens across 8 cores (2048 each).
Each shard of hidden_states is laid out (H=1536, T=2048) host-side so the
contraction dim lands on SBUF partitions (PE matmul contracts over the
partition dim; f32 DMA-transpose doesn't exist on TRN2).  The tiny router
weight (8,1536) is replicated to every core as W^T (1536, 8).

Per core: for each 128-token tile, 12 accumulating PE matmuls
(lhsT = hsT chunk 128x128, rhs = W^T chunk 128x8) produce logits
(128 tokens, 8 experts) in PSUM; ACT applies sigmoid PSUM->SBUF; DVE adds
the expert bias and runs max8/max_index for the sorted top-2; a batched
reciprocal*mul normalizes the top-2 weights; two DMAs write the outputs.
"""

import sys

for _p in ("/opt/trn_rl_repo",):
    if _p not in sys.path:
        sys.path.insert(0, _p)

import numpy as np

import concourse.bacc as bacc
import concourse.mybir as mybir
import concourse.tile as tile
from concourse.bass_utils import run_bass_kernel_spmd

# Problem constants (hardcoded per harness contract)
B, S, H = 4, 4096, 1536
T_FULL = B * S            # 16384 tokens
N_CORES = 8
T = T_FULL // N_CORES     # 2048 tokens per core
E = 8                     # experts
TOP_K = 2
C = H // 128              # 12 contraction chunks
T_BLK = 256               # tokens per DMA block
N_BLK = T // T_BLK        # 8 blocks
TILES_PER_BLK = T_BLK // 128   # 2
N_TILES = T // 128        # 16 tiles of 128 tokens

MM_DTYPE = mybir.dt.float32    # switch to mybir.dt.float32r to test fast path

_CACHE = {}


def _build():
    nc = bacc.Bacc("TRN2", target_bir_lowering=False, debug=False,
                   enable_asserts=False, num_devices=N_CORES)

    hsT = nc.dram_tensor("hsT", [H, T], mybir.dt.float32, kind="ExternalInput").ap()
    wt = nc.dram_tensor("wt", [H, E], mybir.dt.float32, kind="ExternalInput").ap()
    bias = nc.dram_tensor("bias", [1, E], mybir.dt.float32, kind="ExternalInput").ap()
    out_idx = nc.dram_tensor("out_idx", [T, TOP_K], mybir.dt.int32,
                             kind="ExternalOutput").ap()
    out_w = nc.dram_tensor("out_w", [T, TOP_K], mybir.dt.float32,
                           kind="ExternalOutput").ap()

    with tile.TileContext(nc) as tc:
        with (
            tc.tile_pool(name="const", bufs=1) as const_pool,
            tc.tile_pool(name="blk", bufs=3) as blk_pool,
            tc.tile_pool(name="sc", bufs=4) as sc_pool,
            tc.tile_pool(name="acc", bufs=1) as acc_pool,
            tc.tile_pool(name="ps", bufs=4, space="PSUM") as ps_pool,
            tc.tile_pool(name="psb", bufs=1, space="PSUM") as psb_pool,
        ):
            # --- one-time setup ---
            # W^T in SBUF: wt_sb[p, c*8+e] = W[e, 128c+p]
            wt_sb = const_pool.tile([128, C * E], mybir.dt.float32)
            nc.sync.dma_start(wt_sb[:].rearrange("p (c e) -> p c e", e=E),
                              wt.rearrange("(c p) e -> p c e", p=128))

            # bias broadcast to all 128 partitions via ones-outer-product
            bias_sb = const_pool.tile([1, E], mybir.dt.float32)
            nc.sync.dma_start(bias_sb[:], bias[:, :])
            ones_sb = const_pool.tile([1, 128], mybir.dt.float32)
            nc.vector.memset(ones_sb[:], 1.0)
            bias_ps = psb_pool.tile([128, E], mybir.dt.float32)
            nc.tensor.matmul(bias_ps[:], ones_sb[:], bias_sb[:], start=True, stop=True)
            # two copies side by side -> (128, 2*8) so a whole block adds in one op
            bias_bc = const_pool.tile([128, TILES_PER_BLK * E], mybir.dt.float32)
            for j in range(TILES_PER_BLK):
                nc.vector.tensor_copy(bias_bc[:, j * E:(j + 1) * E], bias_ps[:])

            # accumulators across the 16 tiles
            mx_all = acc_pool.tile([128, N_TILES * E], mybir.dt.float32)
            ix_all = acc_pool.tile([128, N_TILES * E], mybir.dt.uint32)

            # --- main pipeline over token blocks ---
            hsT_v = hsT.rearrange("(c p) t -> p c t", p=128)
            for b in range(N_BLK):
                blk = blk_pool.tile([128, C, T_BLK], mybir.dt.float32)
                nc.sync.dma_start(blk[:], hsT_v[:, :, b * T_BLK:(b + 1) * T_BLK])

                ps = ps_pool.tile([128, TILES_PER_BLK, E], mybir.dt.float32)
                for j in range(TILES_PER_BLK):
                    for c in range(C):
                        lhsT = blk[:, c, j * 128:(j + 1) * 128]
                        rhs = wt_sb[:, c * E:(c + 1) * E]
                        if MM_DTYPE != mybir.dt.float32:
                            lhsT = lhsT.bitcast(MM_DTYPE)
                            rhs = rhs.bitcast(MM_DTYPE)
                        nc.tensor.matmul(ps[:, j, :], lhsT, rhs,
                                         start=(c == 0), stop=(c == C - 1))

                # sigmoid (PSUM -> SBUF), then +bias, for the whole block at once
                sc = sc_pool.tile([128, TILES_PER_BLK * E], mybir.dt.float32)
                nc.scalar.activation(sc[:], ps[:],
                                     mybir.ActivationFunctionType.Sigmoid)
                nc.vector.tensor_add(sc[:], sc[:], bias_bc[:])

                for j in range(TILES_PER_BLK):
                    i = b * TILES_PER_BLK + j
                    scj = sc[:, j * E:(j + 1) * E]
                    nc.vector.max(out=mx_all[:, i * E:(i + 1) * E], in_=scj)
                    nc.vector.max_index(out=ix_all[:, i * E:(i + 1) * E],
                                        in_max=mx_all[:, i * E:(i + 1) * E],
                                        in_values=scj)

            # --- batched top-2 normalization + output ---
            mx_v = mx_all[:].rearrange("p (i e) -> p i e", e=E)[:, :, 0:TOP_K]
            ix_v = ix_all[:].rearrange("p (i e) -> p i e", e=E)[:, :, 0:TOP_K]

            wsum = acc_pool.tile([128, N_TILES], mybir.dt.float32)
            nc.vector.reduce_sum(wsum[:], mx_v, axis=mybir.AxisListType.X)
            rinv = acc_pool.tile([128, N_TILES], mybir.dt.float32)
            nc.vector.reciprocal(rinv[:], wsum[:])

            w_out = acc_pool.tile([128, N_TILES * TOP_K], mybir.dt.float32)
            w_out_v = w_out[:].rearrange("p (i k) -> p i k", k=TOP_K)
            for k in range(TOP_K):
                nc.vector.tensor_mul(w_out_v[:, :, k], mx_v[:, :, k], rinv[:])

            i_out = acc_pool.tile([128, N_TILES * TOP_K], mybir.dt.int32)
            i_out_v = i_out[:].rearrange("p (i k) -> p i k", k=TOP_K)
            nc.vector.tensor_copy(i_out_v, ix_v)

            nc.sync.dma_start(out_w.rearrange("(i p) k -> p i k", p=128), w_out_v)
            nc.sync.dma_start(out_idx.rearrange("(i p) k -> p i k", p=128), i_out_v)

    nc.compile()
    return nc


def _get_nc():
    if "nc" not in _CACHE:
        _CACHE["nc"] = _build()
    return _CACHE["nc"]


def kernel(hidden_states, weight, e_score_correction_bias, _trace=False):
    nc = _get_nc()

    hs = np.ascontiguousarray(np.asarray(hidden_states, dtype=np.float32)).reshape(
        T_FULL, H)
    wt_np = np.ascontiguousarray(np.asarray(weight, dtype=np.float32).T)
    bias_np = np.ascontiguousarray(
        np.asarray(e_score_correction_bias, dtype=np.float32).reshape(1, E))

    in_maps = []
    for c in range(N_CORES):
        shard = np.ascontiguousarray(hs[c * T:(c + 1) * T, :].T)  # (H, T)
        in_maps.append({"hsT": shard, "wt": wt_np, "bias": bias_np})

    kw = {}
    if _trace:
        kw = dict(trace=True, trace_cores=[0])
    res = run_bass_kernel_spmd(nc, in_maps, core_ids=list(range(N_CORES)), **kw)
    _CACHE["last_result"] = res

    idx = np.concatenate([np.asarray(r["out_idx"]) for r in res.results], axis=0)
    w = np.concatenate([np.asarray(r["out_w"]) for r in res.results], axis=0)
    return idx.astype(np.int32), w.astype(np.float32)
